# revision 1
# baseline (speedup 1.0000x reference)
"""Trainium2 Bass kernel: dense multi-head dot-product attention.

Problem: x [4, 2048, 1024], W_Q/W_K/W_V [16, 1024, 64] ->
         out [4, 2048, 1024] (heads concatenated on the feature dim).

Sharding: 8 cores = 4 batches x 2 head-groups (8 heads each).
Per core, everything is computed in "transposed" layouts so that no
on-chip transpose of the big attention matrix is ever needed:
  - host passes x^T [1024, 2048] (n on partitions) per batch
  - projections (W stationary): Q^T/K^T/V^T [heads*64, 2048]
  - scores S^T[k, m] = sum_d K^T[d,k] Q^T[d,m]  (k on partitions)
  - P^T = exp(S^T/8)  (elementwise, ScalarE, PSUM->SBUF)
  - O^T[d, m] = sum_k Vaug[k, d] P^T[k, m] with Vaug = [V | ones],
    so row 64 of the accumulator is the softmax denominator.
  - normalize: recip(row64) broadcast over partitions (GpSimd), DVE mul
  - output O^T [512, 2048] per core; host transposes when gathering.
Softmax skips the max-subtraction: |S/8| < ~12 here, exp is safe in fp32
and softmax is shift-invariant, so the result is mathematically identical.

Matmul operands are float32r (fp32 bits, PE rounds to a reduced-precision
mode internally, ~1e-4 rel err, ~1 cyc/row at free-dim 512) by default.
PSUM accumulation stays fp32. KERNEL_MM_DTYPE=bf16|f32 to override.

The attention loop is split into m-halves of 1024 so that PSUM fits:
  shared tag (S^T chunks / proj accum / V-transposes)
                     3 bufs x [128,1024] fp32 = 6 banks
  ot (O^T accum)     1 buf  x [65, 1024] fp32 = 2 banks
Three rotating bufs let the PE run two score chunks ahead of ScalarE's
exp and let next-pair projections interleave with current attention.
"""

import os
from contextlib import ExitStack

import numpy as np

import concourse.bass as bass  # noqa: F401  (bass types via bacc)
import concourse.tile as tile
from concourse import bacc, mybir
from concourse import bass_utils
from concourse.masks import make_identity

F32 = mybir.dt.float32
F32R = mybir.dt.float32r
BF16 = mybir.dt.bfloat16

B, M, N, H, D = 4, 2048, 1024, 16, 64
HPC = 8          # heads per core
NCORES = 8
NCH = 8          # d_model / 128 chunks
KC = 16          # key chunks of 128
SCALE = 0.125    # 1/sqrt(64)
MH = 1024        # m-half width

_MM_DT = os.environ.get("KERNEL_MM_DTYPE", "f32r")
TMM = {"bf16": BF16, "f32r": F32R, "f32": F32}[_MM_DT]


def build_nc():
    nc = bacc.Bacc(
        "TRN2", target_bir_lowering=False, debug=False, enable_asserts=False
    )
    xt_d = nc.dram_tensor("xt", [N, M], F32, kind="ExternalInput")
    wq_d = nc.dram_tensor("wq", [4, N, 128], F32, kind="ExternalInput")
    wk_d = nc.dram_tensor("wk", [4, N, 128], F32, kind="ExternalInput")
    wv_d = nc.dram_tensor("wv", [4, N, 128], F32, kind="ExternalInput")
    o_d = nc.dram_tensor("ot", [HPC * D, M], F32, kind="ExternalOutput")

    with tile.TileContext(nc) as tc, ExitStack() as ctx:
        const_pool = ctx.enter_context(tc.tile_pool(name="constp", bufs=1))
        xt_pool = ctx.enter_context(tc.tile_pool(name="xtp", bufs=NCH))
        w_pool = ctx.enter_context(tc.tile_pool(name="wp", bufs=3))
        qkv_pool = ctx.enter_context(tc.tile_pool(name="qkvp", bufs=2))
        vaug_pool = ctx.enter_context(tc.tile_pool(name="vaugp", bufs=2))
        pt_pool = ctx.enter_context(tc.tile_pool(name="ptp", bufs=4))
        out_pool = ctx.enter_context(tc.tile_pool(name="outp", bufs=4))
        small_pool = ctx.enter_context(tc.tile_pool(name="smallp", bufs=3))
        # PSUM: shared tag (st chunks / proj accum / transposes) 3x2 banks
        # + ot 1x2 banks = 8 banks. Three bufs let the PE run two score
        # chunks ahead of ScalarE's exp, hiding the exp latency.
        st_pool = ctx.enter_context(tc.tile_pool(name="stp", bufs=3, space="PSUM"))
        ot_pool = ctx.enter_context(tc.tile_pool(name="otp", bufs=1, space="PSUM"))

        # memset/affine_select cannot target f32r, and transpose operands
        # must share a dtype: keep the V^T -> V transpose in plain f32
        # (the copy into vaug rounds to TMM).
        TID = BF16 if TMM == BF16 else F32
        ident = const_pool.tile([128, 128], TID, name="ident")
        make_identity(nc, ident[:])
        ones16 = const_pool.tile([128, 16, 1], F32, name="ones16")
        nc.gpsimd.memset(ones16[:], 1.0)

        # ---- resident x^T tiles; SWDGE DMA casts fp32 -> TMM on load
        # load in m-quarters, first quarter for all chunks first, so the
        # first projection matmuls start after 2MB instead of 8MB.
        xts = []
        for c in range(NCH):
            xtile = xt_pool.tile([128, M], TMM, name=f"xt{c}", tag="xtile")
            nc.gpsimd.dma_start(
                xtile[:, 0:512], xt_d.ap()[c * 128:(c + 1) * 128, 0:512]
            )
            xts.append(xtile)
        for q in range(1, 4):
            for c in range(NCH):
                nc.gpsimd.dma_start(
                    xts[c][:, q * 512:(q + 1) * 512],
                    xt_d.ap()[c * 128:(c + 1) * 128, q * 512:(q + 1) * 512],
                )

        for p in range(4):  # head pairs
            # ---- projections: dst[h%2*64+d, m] for the two heads of pair p
            qkv = {}
            for nm, wd in (("q", wq_d), ("k", wk_d), ("v", wv_d)):
                wt = w_pool.tile([128, NCH, 128], TMM, name=f"wt_{nm}", tag="wt")
                nc.gpsimd.dma_start(
                    wt[:], wd.ap()[p].rearrange("(c p) d -> p c d", p=128)
                )
                ddt = TMM if nm in ("q", "k") else TID
                dst = qkv_pool.tile([128, M], ddt, name=f"{nm}t", tag=f"{nm}t")
                for mh in range(2):
                    ps = st_pool.tile([128, MH], F32, name="ps_prj", tag="st")
                    for c in range(NCH):
                        for mc in range(2):
                            nc.tensor.matmul(
                                ps[:, mc * 512:(mc + 1) * 512],
                                lhsT=wt[:, c, :],
                                rhs=xts[c][
                                    :,
                                    mh * MH + mc * 512: mh * MH + (mc + 1) * 512,
                                ],
                                start=(c == 0),
                                stop=(c == NCH - 1),
                                skip_group_check=True,
                            )
                    nc.vector.tensor_copy(dst[:, mh * MH:(mh + 1) * MH], ps[:])
                qkv[nm] = dst
            qt, kt, vt = qkv["q"], qkv["k"], qkv["v"]

            # ---- Vaug[k, kc, hp*65 + d]; col hp*65+64 = 1.0 (ones col)
            vaug = vaug_pool.tile([128, KC, 130], TMM, name="vaug", tag="vaug")
            for hp in range(2):
                nc.vector.tensor_copy(
                    vaug[:, :, hp * 65 + 64:hp * 65 + 65], ones16[:]
                )
            for kc in range(KC):
                trp = st_pool.tile([128, 128], TID, name="trp", tag="st")
                nc.tensor.transpose(
                    trp[:], vt[:, kc * 128:(kc + 1) * 128], ident[:]
                )
                nc.vector.tensor_copy(
                    vaug[:, kc, :].rearrange("p (h x) -> p h x", h=2)[:, :, 0:64],
                    trp.rearrange("p (h d) -> p h d", h=2),
                )

            # ---- attention per head, split in m-halves of 1024
            for hp in range(2):
                h = 2 * p + hp
                hsl = slice(64 * hp, 64 * (hp + 1))
                for mh in range(2):
                    mbase = mh * MH
                    ot = ot_pool.tile([65, MH], F32, name="ot", tag="ot")
                    for kc in range(KC):
                        st = st_pool.tile([128, MH], F32, name="st", tag="st")
                        for mc in range(2):
                            nc.tensor.matmul(
                                st[:, mc * 512:(mc + 1) * 512],
                                lhsT=kt[hsl, kc * 128:(kc + 1) * 128],
                                rhs=qt[
                                    hsl,
                                    mbase + mc * 512: mbase + (mc + 1) * 512,
                                ],
                                start=True,
                                stop=True,
                            )
                        pt = pt_pool.tile([128, MH], TMM, name="pt", tag="pt")
                        nc.scalar.activation(
                            pt[:], st[:],
                            mybir.ActivationFunctionType.Exp, scale=SCALE,
                        )
                        for mc in range(2):
                            nc.tensor.matmul(
                                ot[:, mc * 512:(mc + 1) * 512],
                                lhsT=vaug[:, kc, hp * 65:hp * 65 + 65],
                                rhs=pt[:, mc * 512:(mc + 1) * 512],
                                start=(kc == 0),
                                stop=(kc == KC - 1),
                                skip_group_check=True,
                            )
                    # ---- normalize rows 0..63 by row 64; free ot ASAP
                    sumsb = small_pool.tile([1, MH], F32, name="sumsb", tag="sm")
                    nc.vector.tensor_copy(sumsb[:], ot[64:65, :])
                    ostage = out_pool.tile([64, MH], F32, name="ostage", tag="o64")
                    nc.vector.tensor_copy(ostage[:], ot[0:64, :])
                    recipb = small_pool.tile([1, MH], F32, name="recipb", tag="sm")
                    scratch = small_pool.tile([1, MH], F32, name="scr", tag="sm")
                    nc.vector.reciprocal_approx_accurate(
                        recipb[:], sumsb[:], scratch[:]
                    )
                    rbc = out_pool.tile([64, MH], F32, name="rbc", tag="o64")
                    nc.gpsimd.partition_broadcast(rbc[:], recipb[:])
                    stage = out_pool.tile([64, MH], F32, name="stage", tag="o64")
                    nc.vector.tensor_mul(stage[:], ostage[:], rbc[:])
                    nc.sync.dma_start(
                        o_d.ap()[h * 64:(h + 1) * 64, mbase:mbase + MH], stage[:]
                    )
    nc.compile()
    return nc


_NC_CACHE = None


def _get_nc():
    global _NC_CACHE
    if _NC_CACHE is None:
        _NC_CACHE = build_nc()
    return _NC_CACHE


def make_in_maps(x, W_Q, W_K, W_V):
    x = np.asarray(x, dtype=np.float32)
    W_Q = np.asarray(W_Q, dtype=np.float32)
    W_K = np.asarray(W_K, dtype=np.float32)
    W_V = np.asarray(W_V, dtype=np.float32)

    def prep_w(W, g):
        blk = W[8 * g:8 * g + 8]  # [8, 1024, 64]
        # pair-major [4, 1024, 128]: col = (head%2)*64 + d
        return np.ascontiguousarray(
            blk.reshape(4, 2, N, D).transpose(0, 2, 1, 3).reshape(4, N, 2 * D)
        )

    in_maps = []
    for c in range(NCORES):
        b, g = divmod(c, 2)
        in_maps.append(
            {
                "xt": np.ascontiguousarray(x[b].T),
                "wq": prep_w(W_Q, g),
                "wk": prep_w(W_K, g),
                "wv": prep_w(W_V, g),
            }
        )
    return in_maps


def gather_out(results):
    out = np.empty((B, M, N), dtype=np.float32)
    for c in range(NCORES):
        b, g = divmod(c, 2)
        out[b, :, 512 * g:512 * (g + 1)] = results[c]["ot"].T
    return out


def run(x, W_Q, W_K, W_V, **spmd_kwargs):
    nc = _get_nc()
    in_maps = make_in_maps(x, W_Q, W_K, W_V)
    res = bass_utils.run_bass_kernel_spmd(
        nc, in_maps, core_ids=list(range(NCORES)), **spmd_kwargs
    )
    return gather_out(res.results), res


def kernel(x, W_Q, W_K, W_V):
    out, _ = run(x, W_Q, W_K, W_V)
    return out



# revision 19
# speedup vs baseline: 1.1316x; 1.1316x over previous
"""Trainium2 Bass kernel: dense multi-head dot-product attention.

Problem: x [4, 2048, 1024], W_Q/W_K/W_V [16, 1024, 64] ->
         out [4, 2048, 1024] (heads concatenated on the feature dim).

Sharding: 8 cores = 4 batches x 2 head-groups (8 heads each).
Per core, everything is computed in "transposed" layouts so that no
on-chip transpose of the big attention matrix is ever needed:
  - host passes x^T [1024, 2048] (n on partitions) per batch
  - Q^T/K^T/V^T [2*64, 2048] per head pair (W stationary, x^T moving)
  - Vaug[k, kc, hp*65+d] built from V^T via PE transposes; col hp*65+64
    holds ones so row 64 of the PV accumulator is the softmax denom.
  - scores S^T[k, m] = sum_d K^T[d,k] Q^T[d,m]  (k on partitions).
    The two heads of a pair use PE row groups {0,1} / {2,3}
    (contraction rows 0-63 / 64-127) so their score matmuls run
    CONCURRENTLY in the systolic array (row tiling).
  - P^T = exp(S^T/8): one ScalarE activation per (kc, mq) covering
    both heads (N=1024). ScalarE is the critical engine (~33.5M exps
    per core ~= 260us); the schedule is built to never starve it.
    Softmax skips max-subtraction: |S/8| < ~12, exp safe in fp32,
    softmax shift-invariant.
  - O^T accumulated in PSUM over the 16 key chunks per head.
  - normalize: recip (DVE) -> partition_broadcast (GpSimd) -> mul
    (DVE) -> DMA out. Host transposes when gathering.

Schedule: all weights are prefetched to SBUF at kernel start; the
projection work for pair p+1 is emitted as small bursts interleaved
into pair p's attention kc-loop, so the PE's spare cycles inside the
ScalarE-gated steady state do the projection work and ScalarE never
idles between pairs.

Matmul operands are bf16 (PSUM accumulation fp32): measured ~1.1e-2
rel err vs the 2e-2 gate. fp32/f32r matmuls lower to fp32_mode=HIGH
(half rate) on this stack, so bf16 is the fast path.

PSUM budget (8 banks): shared ring 3 bufs x [128,2,512] fp32 (2 banks
each; scores / proj accum / transposes) + 2 x ot [65,512] (1 bank
each).
"""

import os
from contextlib import ExitStack

import numpy as np

import concourse.bass as bass  # noqa: F401  (bass types via bacc)
import concourse.tile as tile
from concourse import bacc, mybir
from concourse import bass_utils
from concourse.masks import make_identity

F32 = mybir.dt.float32
BF16 = mybir.dt.bfloat16

B, M, N, H, D = 4, 2048, 1024, 16, 64
NCORES = 8
NCH = 8          # d_model / 128 chunks
KC = 16          # key chunks of 128
MQ = 4           # m blocks of 512
SCALE = 0.125    # 1/sqrt(64)
EXPF = mybir.ActivationFunctionType.Exp


def build_nc():
    nc = bacc.Bacc(
        "TRN2", target_bir_lowering=False, debug=False, enable_asserts=False
    )
    xt_d = nc.dram_tensor("xt", [N, M], F32, kind="ExternalInput")
    wq_d = nc.dram_tensor("wq", [4, N, 128], F32, kind="ExternalInput")
    wk_d = nc.dram_tensor("wk", [4, N, 128], F32, kind="ExternalInput")
    wv_d = nc.dram_tensor("wv", [4, N, 128], F32, kind="ExternalInput")
    o_d = nc.dram_tensor("ot", [8 * D, M], F32, kind="ExternalOutput")
    _dbg = bool(os.environ.get("KERNEL_DEBUG_DUMP"))
    if _dbg:
        qt_dbg = nc.dram_tensor("qt_dbg", [128, M], F32, kind="ExternalOutput")
        kt_dbg = nc.dram_tensor("kt_dbg", [128, M], F32, kind="ExternalOutput")
        va_dbg = nc.dram_tensor("va_dbg", [128, KC * 130], F32, kind="ExternalOutput")
    _dbg2 = bool(os.environ.get("KERNEL_DEBUG_OSB"))
    if _dbg2:
        osb_dbg = nc.dram_tensor("osb_dbg", [8, 65, 512], F32, kind="ExternalOutput")
        stg_dbg = nc.dram_tensor("stg_dbg", [8, 64, 512], F32, kind="ExternalOutput")

    with tile.TileContext(nc) as tc, ExitStack() as ctx:
        const_pool = ctx.enter_context(tc.tile_pool(name="constp", bufs=1))
        xt_pool = ctx.enter_context(tc.tile_pool(name="xtp", bufs=NCH))
        w_pool = ctx.enter_context(tc.tile_pool(name="wp", bufs=12))
        qkv_pool = ctx.enter_context(tc.tile_pool(name="qkvp", bufs=2))
        vaug_pool = ctx.enter_context(tc.tile_pool(name="vaugp", bufs=2))
        pt_pool = ctx.enter_context(tc.tile_pool(name="ptp", bufs=4))
        out_pool = ctx.enter_context(tc.tile_pool(name="outp", bufs=8))
        small_pool = ctx.enter_context(tc.tile_pool(name="smallp", bufs=6))
        # PSUM: shared ring (scores / proj accum / transposes) 3 x 2 banks
        # + 2 x ot accumulators (1 bank each) = 8 banks.
        st_pool = ctx.enter_context(tc.tile_pool(name="stp", bufs=3, space="PSUM"))
        ot_pool = ctx.enter_context(tc.tile_pool(name="otp", bufs=2, space="PSUM"))

        ident = const_pool.tile([128, 128], BF16, name="ident")
        make_identity(nc, ident[:])
        ones16 = const_pool.tile([128, KC, 1], F32, name="ones16")
        nc.gpsimd.memset(ones16[:], 1.0)

        # ---- resident x^T tiles (bf16; SWDGE casts fp32 on load).
        # First m-quarter for all chunks first so pair-0 projections can
        # start after 2MB of DMA instead of 8MB.
        xts = []
        for c in range(NCH):
            xtile = xt_pool.tile([128, M], BF16, name=f"xt{c}", tag="xtile")
            nc.gpsimd.dma_start(
                xtile[:, 0:512], xt_d.ap()[c * 128:(c + 1) * 128, 0:512]
            )
            xts.append(xtile)

        # ---- prefetch ALL weights (pair-major [128, 8, 128] bf16)
        wts = {}
        for nm, wd in (("q", wq_d), ("k", wk_d), ("v", wv_d)):
            for p in range(4):
                wt = w_pool.tile([128, NCH, 128], BF16, name=f"w{nm}{p}", tag="wt")
                nc.gpsimd.dma_start(
                    wt[:], wd.ap()[p].rearrange("(c p) d -> p c d", p=128)
                )
                wts[(nm, p)] = wt

        for q in range(1, 4):
            for c in range(NCH):
                nc.gpsimd.dma_start(
                    xts[c][:, q * 512:(q + 1) * 512],
                    xt_d.ap()[c * 128:(c + 1) * 128, q * 512:(q + 1) * 512],
                )

        # ---------------------------------------------------------------
        # Projection work for pair p as a list of small emission bursts
        # (interleaved into the attention loop; each <= ~3.5us of PE so
        # the score ring keeps ScalarE fed). Same instruction patterns
        # as the proven baseline: LDW,MM,MM per chunk; V^T + transposes.
        # ---------------------------------------------------------------
        def make_pair_tiles(p):
            qt = qkv_pool.tile([128, M], BF16, name="qt", tag="qt")
            kt = qkv_pool.tile([128, M], BF16, name="kt", tag="kt")
            vt = qkv_pool.tile([128, M], BF16, name="vt", tag="vt")
            vaug = vaug_pool.tile([128, KC, 130], BF16, name="vaug", tag="vaug")
            return qt, kt, vt, vaug

        def proj_bursts(p, qt, kt, vt, vaug):
            bursts = []

            def ones_burst():
                for hp in range(2):
                    nc.vector.tensor_copy(
                        vaug[:, :, hp * 65 + 64:hp * 65 + 65], ones16[:]
                    )
            bursts.append(ones_burst)

            # Q/K/V^T projections in m-halves of 1024 (16 MMs per burst)
            for nm, dst in (("q", qt), ("k", kt), ("v", vt)):
                wt = wts[(nm, p)]
                for mh in range(2):
                    def qk_burst(wt=wt, dst=dst, mh=mh):
                        ps = st_pool.tile([128, 2, 512], F32, name="ps_prj", tag="st")
                        for c in range(NCH):
                            for mc in range(2):
                                nc.tensor.matmul(
                                    ps[:, mc, :],
                                    lhsT=wt[:, c, :],
                                    rhs=xts[c][
                                        :,
                                        mh * 1024 + mc * 512:
                                        mh * 1024 + (mc + 1) * 512,
                                    ],
                                    start=(c == 0),
                                    stop=(c == NCH - 1),
                                    skip_group_check=True,
                                )
                        nc.vector.tensor_copy(
                            dst[:, mh * 1024:(mh + 1) * 1024].rearrange(
                                "p (c x) -> p c x", c=2
                            ),
                            ps[:],
                        )
                    bursts.append(qk_burst)

            # V^T -> Vaug via PE transposes, 4 kc per burst
            for kg in range(4):
                def v_burst(kg=kg):
                    for k4 in range(4):
                        kc = kg * 4 + k4
                        trp = st_pool.tile([128, 128], BF16, name="trp", tag="st")
                        nc.tensor.transpose(
                            trp[:], vt[:, kc * 128:(kc + 1) * 128], ident[:]
                        )
                        nc.vector.tensor_copy(
                            vaug[:, kc, :].rearrange(
                                "p (h x) -> p h x", h=2
                            )[:, :, 0:64],
                            trp.rearrange("p (h d) -> p h d", h=2),
                        )
                bursts.append(v_burst)
            return bursts

        # Deferred ot drain: the DVE copy out of the PV accumulator is
        # emitted one mq later, gated (via DVE program order) on the next
        # mq's first exp output, so it can never race the tail of the
        # PV-stop matmul's systolic drain into PSUM.
        _PENDING_DRAIN = [None]

        def emit_drain(ots, p, mq, gate_pt=None):
            msl = slice(mq * 512, (mq + 1) * 512)
            if gate_pt is not None:
                gate = small_pool.tile([1, 8], F32, name="gate", tag="gate")
                if len(gate_pt.shape) == 3:
                    nc.vector.tensor_copy(gate[:], gate_pt[0:1, 0, 0:8])
                else:
                    nc.vector.tensor_copy(gate[:], gate_pt[0:1, 0:8])
            for hp in range(2):
                h = 2 * p + hp
                sumsb = small_pool.tile([1, 512], F32, name="sumsb", tag="sm")
                nc.vector.tensor_copy(sumsb[:], ots[hp][64:65, :])
                ostage = out_pool.tile([64, 512], F32, name="ostage", tag="o64")
                nc.vector.tensor_copy(ostage[:], ots[hp][0:64, :])
                recipb = small_pool.tile([1, 512], F32, name="recipb", tag="sm")
                scratch = small_pool.tile([1, 512], F32, name="scr", tag="sm")
                nc.vector.reciprocal_approx_accurate(
                    recipb[:], sumsb[:], scratch[:]
                )
                rbc = out_pool.tile([64, 512], F32, name="rbc", tag="o64")
                nc.gpsimd.partition_broadcast(rbc[:], recipb[:])
                stage = out_pool.tile([64, 512], F32, name="stage", tag="o64")
                nc.vector.tensor_mul(stage[:], ostage[:], rbc[:])
                nc.sync.dma_start(o_d.ap()[h * 64:(h + 1) * 64, msl], stage[:])
                if _dbg2 and p == 0:
                    nc.sync.dma_start(osb_dbg.ap()[mq * 2 + hp], ostage[:])
                    nc.sync.dma_start(stg_dbg.ap()[mq * 2 + hp], stage[:])

        # ---- pair 0 projections up front (overlaps the x/W DMA)
        cur = make_pair_tiles(0)
        for b in proj_bursts(0, *cur):
            b()

        for p in range(4):
            qt, kt, vt, vaug = cur
            if p < 3:
                nxt = make_pair_tiles(p + 1)
                pending = proj_bursts(p + 1, *nxt)
            else:
                nxt, pending = None, []
            if os.environ.get("KERNEL_NO_INTERLEAVE"):
                for _b in pending:
                    _b()
                pending = []
            # spread next-pair bursts evenly over the 64 kc-iterations
            nburst = len(pending)
            slots = {
                int(round((i + 1) * (MQ * KC) / (nburst + 1))): i
                for i in range(nburst)
            }

            if _dbg and p == 0:
                for dbg_d, src in ((qt_dbg, qt), (kt_dbg, kt)):
                    for blk in range(4):
                        dcp = out_pool.tile([128, 512], F32, name="dcp", tag="dbg")
                        nc.vector.tensor_copy(
                            dcp[:], src[:, blk * 512:(blk + 1) * 512]
                        )
                        nc.sync.dma_start(
                            dbg_d.ap()[:, blk * 512:(blk + 1) * 512], dcp[:]
                        )
                for blk in range(4):
                    dcp = out_pool.tile([128, 4 * 130], F32, name="dcpv", tag="dbg")
                    nc.vector.tensor_copy(
                        dcp[:].rearrange("p (c x) -> p c x", c=4),
                        vaug[:, blk * 4:(blk + 1) * 4, :],
                    )
                    nc.sync.dma_start(
                        va_dbg.ap()[:, blk * 4 * 130:(blk + 1) * 4 * 130], dcp[:]
                    )

            it = 0
            for mq in range(MQ):
                msl = slice(mq * 512, (mq + 1) * 512)
                ot0 = ot_pool.tile([65, 512], F32, name="ot0", tag="ot")
                ot1 = ot_pool.tile([65, 512], F32, name="ot1", tag="ot")
                ots = (ot0, ot1)
                for kc in range(KC):
                    ksl = slice(kc * 128, (kc + 1) * 128)
                    if os.environ.get("KERNEL_SERIAL_HEADS"):
                        for hp in range(2):
                            hsl = slice(64 * hp, 64 * (hp + 1))
                            sth = st_pool.tile([128, 512], F32, name="sth", tag="st")
                            nc.tensor.matmul(
                                sth[:], lhsT=kt[hsl, ksl], rhs=qt[hsl, msl],
                                start=True, stop=True,
                            )
                            pth = pt_pool.tile([128, 512], BF16, name="pth", tag="pt")
                            nc.scalar.activation(pth[:], sth[:], EXPF, scale=SCALE)
                            if kc == 0 and hp == 0 and _PENDING_DRAIN[0] is not None:
                                emit_drain(*_PENDING_DRAIN[0], gate_pt=pth)
                                _PENDING_DRAIN[0] = None
                            nc.tensor.matmul(
                                ots[hp][:],
                                lhsT=vaug[:, kc, hp * 65:hp * 65 + 65],
                                rhs=pth[:],
                                start=(kc == 0),
                                stop=(kc == KC - 1),
                                skip_group_check=True,
                            )
                        if it in slots:
                            pending[slots[it]]()
                        it += 1
                        continue
                    st = st_pool.tile([128, 2, 512], F32, name="st", tag="st")
                    for hp in range(2):
                        hsl = slice(64 * hp, 64 * (hp + 1))
                        nc.tensor.matmul(
                            st[:, hp, :],
                            lhsT=kt[hsl, ksl],
                            rhs=qt[hsl, msl],
                            start=True,
                            stop=True,
                        )
                    pt = pt_pool.tile([128, 2, 512], BF16, name="pt", tag="pt")
                    nc.scalar.activation(pt[:], st[:], EXPF, scale=SCALE)
                    if kc == 0 and _PENDING_DRAIN[0] is not None:
                        emit_drain(*_PENDING_DRAIN[0], gate_pt=pt)
                        _PENDING_DRAIN[0] = None
                    for hp in range(2):
                        nc.tensor.matmul(
                            ots[hp][:],
                            lhsT=vaug[:, kc, hp * 65:hp * 65 + 65],
                            rhs=pt[:, hp, :],
                            start=(kc == 0),
                            stop=(kc == KC - 1),
                            skip_group_check=True,
                        )
                    if it in slots:
                        pending[slots[it]]()
                    it += 1
                _PENDING_DRAIN[0] = (ots, p, mq)
            cur = nxt
        # flush the last mq's drain (no later pt to gate on)
        if _PENDING_DRAIN[0] is not None:
            emit_drain(*_PENDING_DRAIN[0], gate_pt=None)
            _PENDING_DRAIN[0] = None
    nc.compile()
    return nc


_NC_CACHE = None


def _get_nc():
    global _NC_CACHE
    if _NC_CACHE is None:
        _NC_CACHE = build_nc()
    return _NC_CACHE


def make_in_maps(x, W_Q, W_K, W_V):
    x = np.asarray(x, dtype=np.float32)
    W_Q = np.asarray(W_Q, dtype=np.float32)
    W_K = np.asarray(W_K, dtype=np.float32)
    W_V = np.asarray(W_V, dtype=np.float32)

    def prep_w(W, g):
        blk = W[8 * g:8 * g + 8]  # [8, 1024, 64]
        # pair-major [4, 1024, 128]: col = (head%2)*64 + d
        return np.ascontiguousarray(
            blk.reshape(4, 2, N, D).transpose(0, 2, 1, 3).reshape(4, N, 2 * D)
        )

    in_maps = []
    for c in range(NCORES):
        b, g = divmod(c, 2)
        in_maps.append(
            {
                "xt": np.ascontiguousarray(x[b].T),
                "wq": prep_w(W_Q, g),
                "wk": prep_w(W_K, g),
                "wv": prep_w(W_V, g),
            }
        )
    return in_maps


def gather_out(results):
    out = np.empty((B, M, N), dtype=np.float32)
    for c in range(NCORES):
        b, g = divmod(c, 2)
        out[b, :, 512 * g:512 * (g + 1)] = results[c]["ot"].T
    return out


def run(x, W_Q, W_K, W_V, **spmd_kwargs):
    nc = _get_nc()
    in_maps = make_in_maps(x, W_Q, W_K, W_V)
    res = bass_utils.run_bass_kernel_spmd(
        nc, in_maps, core_ids=list(range(NCORES)), **spmd_kwargs
    )
    return gather_out(res.results), res


def kernel(x, W_Q, W_K, W_V):
    out, _ = run(x, W_Q, W_K, W_V)
    return out


# revision 24
# speedup vs baseline: 1.2692x; 1.1216x over previous
"""Trainium2 Bass kernel: dense multi-head dot-product attention.

Problem: x [4, 2048, 1024], W_Q/W_K/W_V [16, 1024, 64] ->
         out [4, 2048, 1024] (heads concatenated on the feature dim).

Sharding: 8 cores = 4 batches x 2 head-groups (8 heads each).
Per core, everything is computed in "transposed" layouts so that no
on-chip transpose of the big attention matrix is ever needed:
  - host passes x^T [1024, 2048] (n on partitions) per batch
  - Q^T/K^T/V^T [2*64, 2048] per head pair (W stationary, x^T moving)
  - Vaug[k, kc, hp*65+d] built from V^T via PE transposes; col hp*65+64
    holds ones so row 64 of the PV accumulator is the softmax denom.
  - scores S^T[k, m] = sum_d K^T[d,k] Q^T[d,m]  (k on partitions).
    The two heads of a pair use PE row groups {0,1} / {2,3}
    (contraction rows 0-63 / 64-127) so their score matmuls run
    CONCURRENTLY in the systolic array (row tiling).
  - P^T = exp(S^T/8): one ScalarE activation per (kc, mq) covering
    both heads (N=1024). ScalarE is the critical engine (~33.5M exps
    per core ~= 260us); the schedule is built to never starve it.
    Softmax skips max-subtraction: |S/8| < ~12, exp safe in fp32,
    softmax shift-invariant.
  - O^T accumulated in PSUM over the 16 key chunks per head.
  - normalize: recip (DVE) -> partition_broadcast (GpSimd) -> mul
    (DVE) -> DMA out. Host transposes when gathering.

Schedule: all weights are prefetched to SBUF at kernel start; the
projection work for pair p+1 is emitted as small bursts interleaved
into pair p's attention kc-loop, so the PE's spare cycles inside the
ScalarE-gated steady state do the projection work and ScalarE never
idles between pairs.

Matmul operands are bf16 (PSUM accumulation fp32): measured ~1.1e-2
rel err vs the 2e-2 gate. fp32/f32r matmuls lower to fp32_mode=HIGH
(half rate) on this stack, so bf16 is the fast path.

PSUM budget (8 banks): shared ring 3 bufs x [128,2,512] fp32 (2 banks
each; scores / proj accum / transposes) + 2 x ot [65,512] (1 bank
each).
"""

import os
from contextlib import ExitStack

import numpy as np

import concourse.bass as bass  # noqa: F401  (bass types via bacc)
import concourse.tile as tile
from concourse import bacc, mybir
from concourse import bass_utils
from concourse.masks import make_identity

F32 = mybir.dt.float32
BF16 = mybir.dt.bfloat16

B, M, N, H, D = 4, 2048, 1024, 16, 64
NCORES = 8
NCH = 8          # d_model / 128 chunks
KC = 16          # key chunks of 128
MQ = 4           # m blocks of 512
SCALE = 0.125    # 1/sqrt(64)
EXPF = mybir.ActivationFunctionType.Exp


def build_nc():
    nc = bacc.Bacc(
        "TRN2", target_bir_lowering=False, debug=False, enable_asserts=False
    )
    xt_d = nc.dram_tensor("xt", [N, M], BF16, kind="ExternalInput")
    wq_d = nc.dram_tensor("wq", [4, N, 128], BF16, kind="ExternalInput")
    wk_d = nc.dram_tensor("wk", [4, N, 128], BF16, kind="ExternalInput")
    wv_d = nc.dram_tensor("wv", [4, N, 128], BF16, kind="ExternalInput")
    o_d = nc.dram_tensor("ot", [8 * D, M], F32, kind="ExternalOutput")
    _dbg = bool(os.environ.get("KERNEL_DEBUG_DUMP"))
    if _dbg:
        qt_dbg = nc.dram_tensor("qt_dbg", [128, M], F32, kind="ExternalOutput")
        kt_dbg = nc.dram_tensor("kt_dbg", [128, M], F32, kind="ExternalOutput")
        va_dbg = nc.dram_tensor("va_dbg", [128, KC * 130], F32, kind="ExternalOutput")
    _dbg2 = bool(os.environ.get("KERNEL_DEBUG_OSB"))
    if _dbg2:
        osb_dbg = nc.dram_tensor("osb_dbg", [8, 65, 512], F32, kind="ExternalOutput")
        stg_dbg = nc.dram_tensor("stg_dbg", [8, 64, 512], F32, kind="ExternalOutput")

    with tile.TileContext(nc) as tc, ExitStack() as ctx:
        const_pool = ctx.enter_context(tc.tile_pool(name="constp", bufs=1))
        xt_pool = ctx.enter_context(tc.tile_pool(name="xtp", bufs=NCH))
        w_pool = ctx.enter_context(tc.tile_pool(name="wp", bufs=12))
        qkv_pool = ctx.enter_context(tc.tile_pool(name="qkvp", bufs=2))
        vaug_pool = ctx.enter_context(tc.tile_pool(name="vaugp", bufs=2))
        pt_pool = ctx.enter_context(tc.tile_pool(name="ptp", bufs=4))
        out_pool = ctx.enter_context(tc.tile_pool(name="outp", bufs=8))
        small_pool = ctx.enter_context(tc.tile_pool(name="smallp", bufs=6))
        # PSUM (8 banks): score ring 2 x [128,2,512] (2 banks each)
        # + proj/transpose accum "ps" 1 bank + 3 x ot (1 bank each).
        st_pool = ctx.enter_context(tc.tile_pool(name="stp", bufs=2, space="PSUM"))
        ot_pool = ctx.enter_context(tc.tile_pool(name="otp", bufs=3, space="PSUM"))

        ident = const_pool.tile([128, 128], BF16, name="ident")
        make_identity(nc, ident[:])
        ones16 = const_pool.tile([128, KC, 1], F32, name="ones16")
        nc.gpsimd.memset(ones16[:], 1.0)

        # ---- DMA order: pair-0 weights, xt quarters 0-1, remaining
        # weights, xt quarters 2-3 — so pair-0 projections start ASAP.
        # All inputs are pre-cast to bf16 on the host (halves DMA bytes).
        wts = {}

        def load_w(nm, p):
            wd = {"q": wq_d, "k": wk_d, "v": wv_d}[nm]
            wt = w_pool.tile([128, NCH, 128], BF16, name=f"w{nm}{p}", tag="wt")
            nc.gpsimd.dma_start(
                wt[:], wd.ap()[p].rearrange("(c p) d -> p c d", p=128)
            )
            wts[(nm, p)] = wt

        for nm in ("q", "k", "v"):
            load_w(nm, 0)
        xts = []
        for c in range(NCH):
            xtile = xt_pool.tile([128, M], BF16, name=f"xt{c}", tag="xtile")
            nc.gpsimd.dma_start(
                xtile[:, 0:512], xt_d.ap()[c * 128:(c + 1) * 128, 0:512]
            )
            xts.append(xtile)
        for c in range(NCH):
            nc.gpsimd.dma_start(
                xts[c][:, 512:1024], xt_d.ap()[c * 128:(c + 1) * 128, 512:1024]
            )
        for p in range(1, 4):
            for nm in ("q", "k", "v"):
                load_w(nm, p)
        for q in range(2, 4):
            for c in range(NCH):
                nc.gpsimd.dma_start(
                    xts[c][:, q * 512:(q + 1) * 512],
                    xt_d.ap()[c * 128:(c + 1) * 128, q * 512:(q + 1) * 512],
                )

        # ---------------------------------------------------------------
        # Projection work for pair p as a list of small emission bursts
        # (interleaved into the attention loop; each <= ~3.5us of PE so
        # the score ring keeps ScalarE fed). Same instruction patterns
        # as the proven baseline: LDW,MM,MM per chunk; V^T + transposes.
        # ---------------------------------------------------------------
        def make_pair_tiles(p):
            qt = qkv_pool.tile([128, M], BF16, name="qt", tag="qt")
            kt = qkv_pool.tile([128, M], BF16, name="kt", tag="kt")
            vt = qkv_pool.tile([128, M], BF16, name="vt", tag="vt")
            vaug = vaug_pool.tile([128, KC, 130], BF16, name="vaug", tag="vaug")
            return qt, kt, vt, vaug

        def proj_bursts(p, qt, kt, vt, vaug):
            bursts = []

            def ones_burst():
                for hp in range(2):
                    nc.vector.tensor_copy(
                        vaug[:, :, hp * 65 + 64:hp * 65 + 65], ones16[:]
                    )
            bursts.append(ones_burst)

            # Q/K/V^T projections in m-blocks of 512, mq-major so early
            # bursts only need the early xt quarters; V^T transposes for
            # a block follow right after its V burst.
            for g in range(4):
                for nm, dst in (("q", qt), ("k", kt), ("v", vt)):
                    def qkv_burst(nm=nm, dst=dst, g=g):
                        wt = wts[(nm, p)]
                        ps = st_pool.tile(
                            [128, 512], F32, name="ps_prj", tag="ps", bufs=1
                        )
                        for c in range(NCH):
                            nc.tensor.matmul(
                                ps[:],
                                lhsT=wt[:, c, :],
                                rhs=xts[c][:, g * 512:(g + 1) * 512],
                                start=(c == 0),
                                stop=(c == NCH - 1),
                                skip_group_check=True,
                            )
                        nc.vector.tensor_copy(
                            dst[:, g * 512:(g + 1) * 512], ps[:]
                        )
                    bursts.append(qkv_burst)

                def tr_burst(g=g):
                    for k4 in range(4):
                        kc = g * 4 + k4
                        trp = st_pool.tile(
                            [128, 128], BF16, name="trp", tag="ps", bufs=1
                        )
                        nc.tensor.transpose(
                            trp[:], vt[:, kc * 128:(kc + 1) * 128], ident[:]
                        )
                        nc.vector.tensor_copy(
                            vaug[:, kc, :].rearrange(
                                "p (h x) -> p h x", h=2
                            )[:, :, 0:64],
                            trp.rearrange("p (h d) -> p h d", h=2),
                        )
                bursts.append(tr_burst)
            return bursts

        # Deferred ot drain: the DVE copy out of the PV accumulator is
        # emitted one mq later, gated (via DVE program order) on the next
        # mq's first exp output, so it can never race the tail of the
        # PV-stop matmul's systolic drain into PSUM.
        _PENDING_DRAIN = [None]

        def emit_drain(ots, p, mq, gate_pt=None):
            msl = slice(mq * 512, (mq + 1) * 512)
            if gate_pt is not None:
                gate = small_pool.tile([1, 8], F32, name="gate", tag="gate")
                if len(gate_pt.shape) == 3:
                    nc.vector.tensor_copy(gate[:], gate_pt[0:1, 0, 0:8])
                else:
                    nc.vector.tensor_copy(gate[:], gate_pt[0:1, 0:8])
            for hp in range(2):
                h = 2 * p + hp
                sumsb = small_pool.tile([1, 512], F32, name="sumsb", tag="sm")
                nc.vector.tensor_copy(sumsb[:], ots[hp][64:65, :])
                ostage = out_pool.tile([64, 512], F32, name="ostage", tag="o64")
                nc.vector.tensor_copy(ostage[:], ots[hp][0:64, :])
                recipb = small_pool.tile([1, 512], F32, name="recipb", tag="sm")
                scratch = small_pool.tile([1, 512], F32, name="scr", tag="sm")
                nc.vector.reciprocal_approx_accurate(
                    recipb[:], sumsb[:], scratch[:]
                )
                rbc = out_pool.tile([64, 512], F32, name="rbc", tag="o64")
                nc.gpsimd.partition_broadcast(rbc[:], recipb[:])
                stage = out_pool.tile([64, 512], F32, name="stage", tag="o64")
                nc.vector.tensor_mul(stage[:], ostage[:], rbc[:])
                nc.sync.dma_start(o_d.ap()[h * 64:(h + 1) * 64, msl], stage[:])
                if _dbg2 and p == 0:
                    nc.sync.dma_start(osb_dbg.ap()[mq * 2 + hp], ostage[:])
                    nc.sync.dma_start(stg_dbg.ap()[mq * 2 + hp], stage[:])

        # ---- pair 0 projections up front (overlaps the x/W DMA)
        cur = make_pair_tiles(0)
        for b in proj_bursts(0, *cur):
            b()

        for p in range(4):
            qt, kt, vt, vaug = cur
            if p < 3:
                nxt = make_pair_tiles(p + 1)
                pending = proj_bursts(p + 1, *nxt)
            else:
                nxt, pending = None, []
            if os.environ.get("KERNEL_NO_INTERLEAVE"):
                for _b in pending:
                    _b()
                pending = []
            # spread next-pair bursts evenly over the 64 kc-iterations
            nburst = len(pending)
            slots = {
                int(round((i + 1) * (MQ * KC) / (nburst + 1))): i
                for i in range(nburst)
            }

            if _dbg and p == 0:
                for dbg_d, src in ((qt_dbg, qt), (kt_dbg, kt)):
                    for blk in range(4):
                        dcp = out_pool.tile([128, 512], F32, name="dcp", tag="dbg")
                        nc.vector.tensor_copy(
                            dcp[:], src[:, blk * 512:(blk + 1) * 512]
                        )
                        nc.sync.dma_start(
                            dbg_d.ap()[:, blk * 512:(blk + 1) * 512], dcp[:]
                        )
                for blk in range(4):
                    dcp = out_pool.tile([128, 4 * 130], F32, name="dcpv", tag="dbg")
                    nc.vector.tensor_copy(
                        dcp[:].rearrange("p (c x) -> p c x", c=4),
                        vaug[:, blk * 4:(blk + 1) * 4, :],
                    )
                    nc.sync.dma_start(
                        va_dbg.ap()[:, blk * 4 * 130:(blk + 1) * 4 * 130], dcp[:]
                    )

            it = 0
            for mq in range(MQ):
                msl = slice(mq * 512, (mq + 1) * 512)
                ot0 = ot_pool.tile([65, 512], F32, name="ot0", tag="ot")
                ot1 = ot_pool.tile([65, 512], F32, name="ot1", tag="ot")
                ots = (ot0, ot1)
                for kc in range(KC):
                    ksl = slice(kc * 128, (kc + 1) * 128)
                    if os.environ.get("KERNEL_SERIAL_HEADS"):
                        for hp in range(2):
                            hsl = slice(64 * hp, 64 * (hp + 1))
                            sth = st_pool.tile([128, 512], F32, name="sth", tag="st")
                            nc.tensor.matmul(
                                sth[:], lhsT=kt[hsl, ksl], rhs=qt[hsl, msl],
                                start=True, stop=True,
                            )
                            pth = pt_pool.tile([128, 512], BF16, name="pth", tag="pt")
                            nc.scalar.activation(pth[:], sth[:], EXPF, scale=SCALE)
                            if kc == 0 and hp == 0 and _PENDING_DRAIN[0] is not None:
                                emit_drain(*_PENDING_DRAIN[0], gate_pt=pth)
                                _PENDING_DRAIN[0] = None
                            nc.tensor.matmul(
                                ots[hp][:],
                                lhsT=vaug[:, kc, hp * 65:hp * 65 + 65],
                                rhs=pth[:],
                                start=(kc == 0),
                                stop=(kc == KC - 1),
                                skip_group_check=True,
                            )
                        if it in slots:
                            pending[slots[it]]()
                        it += 1
                        continue
                    st = st_pool.tile([128, 2, 512], F32, name="st", tag="st")
                    for hp in range(2):
                        hsl = slice(64 * hp, 64 * (hp + 1))
                        nc.tensor.matmul(
                            st[:, hp, :],
                            lhsT=kt[hsl, ksl],
                            rhs=qt[hsl, msl],
                            start=True,
                            stop=True,
                        )
                    pt = pt_pool.tile([128, 2, 512], BF16, name="pt", tag="pt")
                    nc.scalar.activation(pt[:], st[:], EXPF, scale=SCALE)
                    if kc == 0 and _PENDING_DRAIN[0] is not None:
                        emit_drain(*_PENDING_DRAIN[0], gate_pt=pt)
                        _PENDING_DRAIN[0] = None
                    for hp in range(2):
                        nc.tensor.matmul(
                            ots[hp][:],
                            lhsT=vaug[:, kc, hp * 65:hp * 65 + 65],
                            rhs=pt[:, hp, :],
                            start=(kc == 0),
                            stop=(kc == KC - 1),
                            skip_group_check=True,
                        )
                    if it in slots:
                        pending[slots[it]]()
                    it += 1
                _PENDING_DRAIN[0] = (ots, p, mq)
            cur = nxt
        # flush the last mq's drain (no later pt to gate on)
        if _PENDING_DRAIN[0] is not None:
            emit_drain(*_PENDING_DRAIN[0], gate_pt=None)
            _PENDING_DRAIN[0] = None
    nc.compile()
    return nc


_NC_CACHE = None


def _get_nc():
    global _NC_CACHE
    if _NC_CACHE is None:
        _NC_CACHE = build_nc()
    return _NC_CACHE


def make_in_maps(x, W_Q, W_K, W_V):
    import ml_dtypes

    BF = ml_dtypes.bfloat16
    x = np.asarray(x, dtype=np.float32)
    W_Q = np.asarray(W_Q, dtype=np.float32)
    W_K = np.asarray(W_K, dtype=np.float32)
    W_V = np.asarray(W_V, dtype=np.float32)

    def prep_w(W, g):
        blk = W[8 * g:8 * g + 8]  # [8, 1024, 64]
        # pair-major [4, 1024, 128]: col = (head%2)*64 + d
        return np.ascontiguousarray(
            blk.reshape(4, 2, N, D).transpose(0, 2, 1, 3).reshape(4, N, 2 * D)
        ).astype(BF)

    xts = [np.ascontiguousarray(x[b].T).astype(BF) for b in range(B)]
    ws = [
        (prep_w(W_Q, g), prep_w(W_K, g), prep_w(W_V, g)) for g in range(2)
    ]
    in_maps = []
    for c in range(NCORES):
        b, g = divmod(c, 2)
        in_maps.append(
            {
                "xt": xts[b],
                "wq": ws[g][0],
                "wk": ws[g][1],
                "wv": ws[g][2],
            }
        )
    return in_maps


def gather_out(results):
    out = np.empty((B, M, N), dtype=np.float32)
    for c in range(NCORES):
        b, g = divmod(c, 2)
        out[b, :, 512 * g:512 * (g + 1)] = results[c]["ot"].T
    return out


def run(x, W_Q, W_K, W_V, **spmd_kwargs):
    nc = _get_nc()
    in_maps = make_in_maps(x, W_Q, W_K, W_V)
    res = bass_utils.run_bass_kernel_spmd(
        nc, in_maps, core_ids=list(range(NCORES)), **spmd_kwargs
    )
    return gather_out(res.results), res


def kernel(x, W_Q, W_K, W_V):
    out, _ = run(x, W_Q, W_K, W_V)
    return out
